# revision 11
# baseline (speedup 1.0000x reference)
"""MultiHeadLinearAttention (Linformer-style) on 8 trn2 NeuronCores.

Strategy (head-parallel attention + AllToAll + token-parallel output proj):
  - 16 heads -> 8 cores, 2 heads (one d_model slice of 128) per core.
  - Per core, per batch b:
      Kp  [128(d2), 256]    = K_slice^T @ We          (+be)
      Vp  [256(k), 128(d2)] = Wf^T @ V_slice          (+bf)
      s^T [256(k), n]       = Kp_h^T @ Q_h^T  (Q^T prepared on host)
      E^T = exp(s^T / 8)    (softmax without max-subtraction; |s| <= ~6)
      at  [65, n] = [Vp_h | 1]^T @ E^T   (row 64 = softmax denominator)
  - The UNNORMALIZED numerators + denominator rows are evacuated to SBUF in
    one DVE op per (nh, b2) and AllToAll'd (2 halves, b in {0,1} / {2,3}).
  - Post-A2A (token-parallel): per (b): one ACT reciprocal over all 16 head
    denominators, broadcast across the 64 d-dims of each head via a tiny
    ones-matmul on the PE, normalize with 8 DVE mults, then
    out[n_shard] = attn_full^T.T @ Wo (+bo).

DMA-instruction count is the hidden serial cost on this stack (~0.6us of
descriptor generation per dma_start on the issuing engine), so tiles are
sized for >=2KB per partition line and phase-3 loads are issued from the
(otherwise idle there) scalar engine.

All matmuls run in bf16 (inputs host-cast) with fp32 PSUM accumulation.
"""

import numpy as np
import ml_dtypes

import concourse.bass as bass
import concourse.mybir as mybir
from concourse.tile import TileContext
from concourse.bass_utils import run_bass_kernel_spmd

B, N, D, H, LK = 4, 4096, 1024, 16, 256
DK = D // H          # 64
NC = 8               # cores
NSH = N // NC        # 512 tokens per core in phase 3
P = 128
NPH = 8              # 512-col chunks of N
NSB = N // 512       # 8 superblocks of 512 rows (phase-1 K/V interleave)

F32 = mybir.dt.float32
BF16 = mybir.dt.bfloat16
NP_BF16 = ml_dtypes.bfloat16

_BUILD_CACHE = {}

_ws_ctr = [0]


def _split_multi_waits(nc, lim=1):
    """Walrus codegen on this stack rejects instructions whose on_wait list
    exceeds the per-format wait-slot count ("Too many sync wait commands").
    Engines execute in order, so excess waits move onto preceding NOPs on
    the same engine with identical semantics."""
    for f in nc.m.functions:
        for blk in f.blocks:
            insts = blk.instructions
            if not any(
                ins.sync_info is not None and len(ins.sync_info.on_wait or []) > lim
                for ins in insts
            ):
                continue
            out = []
            for ins in insts:
                si = ins.sync_info
                waits = list(si.on_wait) if si is not None and si.on_wait else []
                if len(waits) > lim and ins.engine is not None:
                    keep = waits[-lim:]
                    rest = waits[:-lim]
                    while rest:
                        chunk, rest = rest[:lim], rest[lim:]
                        _ws_ctr[0] += 1
                        nop = mybir.InstNoOp(
                            name=f"I-waitsplit-{_ws_ctr[0]}", ins=[], outs=[]
                        )
                        nop.engine = ins.engine
                        nop.sync_info = mybir.SyncInfo(on_wait=chunk, on_update=[])
                        out.append(nop)
                    ins.sync_info = mybir.SyncInfo(
                        on_wait=keep, on_update=list(si.on_update or [])
                    )
                out.append(ins)
            blk.instructions = out
    return nc


def _build(use_be, use_bf, use_bo):
    nc = bass.Bass(num_devices=NC)

    # host-prepped layouts (see kernel()):  token index n = 512*sb + 4*p + c
    #   Ks/Vs: [8 sb, 128 p, 4 c, B, 128]   (4KB per partition line per tile)
    #   QTs:   [B, 128, N]
    #   WeR/WfR: [128 p, 8 sb, 4 c, 256]
    #   WoR:   [128, 8, 1024]
    Ks_p = nc.declare_dram_parameter("Ks", [NSB, P, 4, B, P], BF16, isOutput=False)
    Vs_p = nc.declare_dram_parameter("Vs", [NSB, P, 4, B, P], BF16, isOutput=False)
    QT_p = nc.declare_dram_parameter("QTs", [B, P, N], BF16, isOutput=False)
    We_p = nc.declare_dram_parameter("WeR", [P, NSB, 4, LK], BF16, isOutput=False)
    Wf_p = nc.declare_dram_parameter("WfR", [P, NSB, 4, LK], BF16, isOutput=False)
    Wo_p = nc.declare_dram_parameter("WoR", [P, D // P, D], BF16, isOutput=False)
    if use_be:
        be_p = nc.declare_dram_parameter("beB", [P, LK], F32, isOutput=False)
    if use_bf:
        bf_p = nc.declare_dram_parameter("bfB", [P, 2], F32, isOutput=False)
    if use_bo:
        bo_p = nc.declare_dram_parameter("boB", [P, D], F32, isOutput=False)
    out_p = nc.declare_dram_parameter("out", [B, NSH, D], F32, isOutput=True)

    rg = [list(range(NC))]

    with TileContext(nc) as tc:
        with (
            tc.tile_pool(name="wpool", bufs=1) as wpool,
            tc.tile_pool(name="state", bufs=1) as state,
            tc.tile_pool(name="dram", bufs=1, space="DRAM") as dram,
        ):
            # ---- resident weights (chunked so phase-1 matmuls start early)
            We_sb = wpool.tile([P, NSB, 4, LK], BF16)
            Wf_sb = wpool.tile([P, NSB, 4, LK], BF16)
            Wo_sb = wpool.tile([P, D // P, D], BF16)
            if use_be:
                be_sb = wpool.tile([P, LK], F32)
                nc.sync.dma_start(be_sb[:], be_p[:])
            if use_bf:
                bf_sb = wpool.tile([P, 2], F32)
                nc.sync.dma_start(bf_sb[:], bf_p[:])

            # ---- A2A buffers, d-major: [dest, 65(attn|den), h, b2, n] bf16
            a2a_in = [
                dram.tile([NC, DK + 1, 2, 2, NSH], BF16, name=f"a2a_in{i}")
                for i in range(2)
            ]
            a2a_out = [
                dram.tile([NC, DK + 1, 2, 2, NSH], BF16, name=f"a2a_out{i}")
                for i in range(2)
            ]

            # persistent per-core attention state
            # Kp_pad[p=d2(zero-padded per head), b, h, kc, 128(k)]
            Kp_pad = state.tile([P, B, 2, 2, P], BF16)
            # Vp_aug[p=k, kc, b, h, 65(d|1)]
            Vp_aug = state.tile([P, 2, B, 2, DK + 1], BF16)
            # E2[p<16, dm, h, 64]: 1 iff p == 8*h + dm  (denominator broadcast
            # weights: rb2[(h,d), n] = E2[:,dm]^T @ rden16; rden row = 8h+c)
            e2f = state.tile([16, NC, 2, DK], F32)
            nc.gpsimd.memset(e2f[:], 0.0)
            nc.gpsimd.affine_select(
                out=e2f[:],
                in_=e2f[:],
                compare_op=mybir.AluOpType.not_equal,
                fill=1.0,
                base=0,
                # val = p - dm - 8*h; fill 1.0 where val == 0
                pattern=[[-1, NC], [-8, 2], [0, DK]],
                channel_multiplier=1,
            )
            E2 = state.tile([16, NC, 2, DK], BF16)
            nc.vector.tensor_copy(E2[:], e2f[:])

            # ================= phase 1: Kp / Vp =================
            with (
                tc.tile_pool(name="p1", bufs=3) as p1,
                tc.tile_pool(name="p1ps", bufs=1, space="PSUM") as p1ps,
            ):
                kp_ps = [
                    p1ps.tile([P, LK], F32, name=f"kp{b}", tag=f"kp{b}")
                    for b in range(B)
                ]
                vp_ps = [
                    p1ps.tile([P, B * P], F32, name=f"vp{kc}", tag=f"vp{kc}")
                    for kc in range(2)
                ]
                for sb in range(NSB):
                    nc.sync.dma_start(We_sb[:, sb, :, :], We_p[:, sb, :, :])
                    nc.sync.dma_start(Wf_sb[:, sb, :, :], Wf_p[:, sb, :, :])
                    K4 = p1.tile([P, 4, B, P], BF16, name="K4", tag="K4")
                    nc.sync.dma_start(K4[:], Ks_p[sb])
                    V4 = p1.tile([P, 4, B, P], BF16, name="V4", tag="V4")
                    nc.sync.dma_start(V4[:], Vs_p[sb])
                    for c in range(4):
                        for b in range(B):
                            nc.tensor.matmul(
                                kp_ps[b][:],
                                K4[:, c, b, :],
                                We_sb[:, sb, c, :],
                                start=(sb == 0 and c == 0),
                                stop=(sb == NSB - 1 and c == 3),
                            )
                        for kc in range(2):
                            nc.tensor.matmul(
                                vp_ps[kc][:],
                                Wf_sb[:, sb, c, kc * P : (kc + 1) * P],
                                V4[:, c, :, :],
                                start=(sb == 0 and c == 0),
                                stop=(sb == NSB - 1 and c == 3),
                            )

                # epilogue: build Kp_pad (zero-padded per head) and Vp_aug
                nc.vector.memset(Kp_pad[:], 0.0)
                nc.vector.memset(Vp_aug[:, :, :, :, DK : DK + 1], 1.0)
                for b in range(B):
                    for h in range(2):
                        hs = slice(h * DK, (h + 1) * DK)
                        for kc in range(2):
                            ks = slice(kc * P, (kc + 1) * P)
                            if use_be:
                                nc.vector.tensor_tensor(
                                    Kp_pad[hs, b, h, kc, :],
                                    kp_ps[b][hs, ks],
                                    be_sb[hs, ks],
                                    mybir.AluOpType.add,
                                )
                            else:
                                nc.vector.tensor_copy(
                                    Kp_pad[hs, b, h, kc, :], kp_ps[b][hs, ks]
                                )
                for kc in range(2):
                    for b in range(B):
                        for h in range(2):
                            src = vp_ps[kc][:, b * P + h * DK : b * P + (h + 1) * DK]
                            dst = Vp_aug[:, kc, b, h, 0:DK]
                            if use_bf:
                                nc.vector.tensor_scalar_add(
                                    dst, src, bf_sb[:, kc : kc + 1]
                                )
                            else:
                                nc.vector.tensor_copy(dst, src)

            # load Wo while phase 2 runs (DMA is idle-ish there)
            nc.sync.dma_start(Wo_sb[:], Wo_p[:])
            if use_bo:
                bo_sb = wpool.tile([P, D], F32)
                nc.sync.dma_start(bo_sb[:], bo_p[:])

            # ========== phase 2: scores/softmax/unnormalized attn + A2A ==========
            with (
                tc.tile_pool(name="p2", bufs=3) as p2,
                tc.tile_pool(name="p2ps", bufs=1, space="PSUM") as p2ps,
            ):
                for half in range(2):
                    QTb = [None, None]
                    for b2 in range(2):
                        b = half * 2 + b2
                        QTb[b2] = p2.tile(
                            [P, N], BF16, name=f"QTb{b2}", tag=f"QTb{b2}", bufs=2
                        )
                        nc.sync.dma_start(QTb[b2][:], QT_p[b])
                    for nh in range(NPH):
                        asb2 = p2.tile(
                            [DK + 1, 2, 2, NSH], BF16, name="asb2", tag="asb2",
                            bufs=3,
                        )
                        for b2 in range(2):
                            b = half * 2 + b2
                            QTn = QTb[b2][:, nh * NSH : (nh + 1) * NSH]
                            st = [
                                p2ps.tile(
                                    [P, 2, NSH], F32, name=f"st{h}", tag=f"st{h}",
                                    bufs=1,
                                )
                                for h in range(2)
                            ]
                            ET = [
                                p2.tile(
                                    [P, 2, NSH], BF16, name=f"ET{h}", tag=f"ET{h}",
                                    bufs=2,
                                )
                                for h in range(2)
                            ]
                            for h in range(2):
                                for kc in range(2):
                                    nc.tensor.matmul(
                                        st[h][:, kc, :],
                                        Kp_pad[:, b, h, kc, :],
                                        QTn,
                                        start=True,
                                        stop=True,
                                    )
                                nc.scalar.activation(
                                    ET[h][:],
                                    st[h][:],
                                    mybir.ActivationFunctionType.Exp,
                                    scale=0.125,
                                )
                            at = p2ps.tile(
                                [DK + 1, 2, NSH], F32, name="at", tag="at", bufs=2
                            )
                            for h in range(2):
                                for kc in range(2):
                                    nc.tensor.matmul(
                                        at[:, h, :],
                                        Vp_aug[:, kc, b, h, :],
                                        ET[h][:, kc, :],
                                        start=(kc == 0),
                                        stop=(kc == 1),
                                    )
                            # single evacuation: numerators + den rows, bf16
                            nc.vector.tensor_copy(asb2[:, :, b2, :], at[:])
                        nc.sync.dma_start(a2a_in[half][nh], asb2[:])
                    nc.gpsimd.collective_compute(
                        "AllToAll",
                        mybir.AluOpType.bypass,
                        replica_groups=rg,
                        ins=[a2a_in[half][:]],
                        outs=[a2a_out[half][:]],
                    )

            # ======= phase 3: normalize + output projection (token-parallel) =======
            # DMA loads here are issued from the scalar engine: the sync
            # engine's serial descriptor generation is the scarce resource
            # and ACT is idle in this phase.
            with (
                tc.tile_pool(name="p3", bufs=3) as p3,
                tc.tile_pool(name="p3ps", bufs=1, space="PSUM") as p3ps,
            ):
                for half in range(2):
                    # raw numerators for both b2 of this half:
                    # g[dm] = [(h d), b2, n]
                    gs = []
                    for dm in range(D // P):
                        g = p3.tile([P, 2, NSH], BF16, name="g", tag="g", bufs=9)
                        for h in range(2):
                            nc.sync.dma_start(
                                g[h * DK : (h + 1) * DK, :, :],
                                a2a_out[half][dm, 0:DK, h, :, :],
                            )
                        gs.append(g)
                    for b2 in range(2):
                        b = half * 2 + b2
                        # denominators for all 16 heads; row = 8h + c
                        den_b = p3.tile([16, NSH], BF16, name="den_b", tag="den_b",
                                        bufs=2)
                        for h in range(2):
                            nc.sync.dma_start(
                                den_b[8 * h : 8 * (h + 1), :],
                                a2a_out[half][:, DK, h, b2, :],
                            )
                        # batched reciprocal: reshape [16,512] -> [128,64]
                        # (row 16j+r <- den_b[r, 64j:64j+64]) so the 8-cyc/elem
                        # DVE divide runs at FD=64, then reshape back.
                        denT = p3.tile([P, DK], BF16, name="denT", tag="denT",
                                       bufs=2)
                        for j in range(8):
                            nc.sync.dma_start(
                                denT[16 * j : 16 * (j + 1), :],
                                den_b[:, DK * j : DK * (j + 1)],
                            )
                        rdenT = p3.tile([P, DK], F32, name="rdenT", tag="rdenT",
                                        bufs=2)
                        nc.vector.reciprocal(rdenT[:], denT[:])
                        rdenT_bf = p3.tile([P, DK], BF16, name="rdenT_bf",
                                           tag="rdenT_bf", bufs=2)
                        nc.vector.tensor_copy(rdenT_bf[:], rdenT[:])
                        rden16 = p3.tile([16, NSH], BF16, name="rden16",
                                         tag="rden16", bufs=2)
                        for j in range(8):
                            nc.sync.dma_start(
                                rden16[:, DK * j : DK * (j + 1)],
                                rdenT_bf[16 * j : 16 * (j + 1), :],
                            )

                        # normalize: rb2[:,dm] = E2[:,dm]^T @ rden16 ; g *= rb2
                        gn = []
                        for dm in range(D // P):
                            rb2 = p3ps.tile([P, NSH], F32, name="rb2", tag="rb2",
                                            bufs=2)
                            nc.tensor.matmul(
                                rb2[:],
                                E2[:, dm, :, :],
                                rden16[:],
                                start=True,
                                stop=True,
                            )
                            g2n = p3.tile([P, NSH], BF16, name="g2n", tag="g2n",
                                          bufs=10)
                            nc.vector.tensor_tensor(
                                g2n[:], gs[dm][:, b2, :], rb2[:],
                                mybir.AluOpType.mult,
                            )
                            gn.append(g2n)

                        for mt in range(NSH // P):
                            f = p3ps.tile(
                                [P, 2, 512], F32, name="f", tag="f", bufs=2
                            )
                            for dm in range(D // P):
                                for fi in range(2):
                                    nc.tensor.matmul(
                                        f[:, fi, :],
                                        gn[dm][:, mt * P : (mt + 1) * P],
                                        Wo_sb[:, dm, fi * 512 : (fi + 1) * 512],
                                        start=(dm == 0),
                                        stop=(dm == D // P - 1),
                                    )
                            osb = p3.tile([P, D], F32, name="osb", tag="osb")
                            if use_bo:
                                nc.vector.tensor_tensor(
                                    osb[:].rearrange("p (f j) -> p f j", f=2),
                                    f[:],
                                    bo_sb[:].rearrange("p (f j) -> p f j", f=2),
                                    mybir.AluOpType.add,
                                )
                            else:
                                nc.scalar.copy(
                                    osb[:].rearrange("p (f j) -> p f j", f=2), f[:]
                                )
                            nc.sync.dma_start(
                                out_p[b, mt * P : (mt + 1) * P, :], osb[:]
                            )

    return nc


def kernel(K, Q, V, We, be, Wf, bf, Wo, bo, n_heads, _trace=False):
    assert int(n_heads) == H
    K = np.asarray(K, np.float32)
    Q = np.asarray(Q, np.float32)
    V = np.asarray(V, np.float32)
    We = np.asarray(We, np.float32)
    be = np.asarray(be, np.float32)
    Wf = np.asarray(Wf, np.float32)
    bf = np.asarray(bf, np.float32)
    Wo = np.asarray(Wo, np.float32)
    bo = np.asarray(bo, np.float32)

    use_be = bool(np.any(be))
    use_bf = bool(np.any(bf))
    use_bo = bool(np.any(bo))

    key = (use_be, use_bf, use_bo)
    if key not in _BUILD_CACHE:
        _BUILD_CACHE[key] = _split_multi_waits(_build(*key))
    nc = _BUILD_CACHE[key]

    Kb = K.astype(NP_BF16)
    Vb = V.astype(NP_BF16)
    Qb = Q.astype(NP_BF16)
    # weight rows follow the same n = 512*sb + 4*p + c interleave as K/V
    WeR = np.ascontiguousarray(
        We.reshape(NSB, P, 4, LK).transpose(1, 0, 2, 3).astype(NP_BF16)
    )
    WfR = np.ascontiguousarray(
        Wf.reshape(NSB, P, 4, LK).transpose(1, 0, 2, 3).astype(NP_BF16)
    )
    WoR = np.ascontiguousarray(
        Wo.reshape(D // P, P, D).transpose(1, 0, 2).astype(NP_BF16)
    )

    in_maps = []
    for c in range(NC):
        cs = slice(P * c, P * (c + 1))
        m = {
            "Ks": np.ascontiguousarray(
                Kb[:, :, cs].transpose(1, 0, 2).reshape(NSB, P, 4, B, P)
            ),
            "Vs": np.ascontiguousarray(
                Vb[:, :, cs].transpose(1, 0, 2).reshape(NSB, P, 4, B, P)
            ),
            "QTs": np.ascontiguousarray(Qb[:, :, cs].transpose(0, 2, 1)),
            "WeR": WeR,
            "WfR": WfR,
            "WoR": WoR,
        }
        if use_be:
            m["beB"] = np.broadcast_to(be, (P, LK)).copy()
        if use_bf:
            m["bfB"] = np.ascontiguousarray(bf.reshape(2, P).T)
        if use_bo:
            m["boB"] = np.broadcast_to(bo, (P, D)).copy()
        in_maps.append(m)

    res = run_bass_kernel_spmd(nc, in_maps, list(range(NC)), trace=_trace)

    out = np.empty((B, N, D), np.float32)
    for c in range(NC):
        out[:, NSH * c : NSH * (c + 1), :] = res.results[c]["out"]
    if _trace:
        kernel._last_exec_time_ns = res.exec_time_ns
    return out


kernel._last_exec_time_ns = None
